# revision 12
# baseline (speedup 1.0000x reference)
"""Trainium2 Bass kernel for nn_CrossAttentionFusion (V=3, B=8192, H=2048, NH=16).

Measured: 2.943 ms HW exec (vs 4.779 ms fp32r baseline), rel err 3.9e-3.
PE-bound at the bf16 1-col/cycle matmul rate with ~97% occupancy; the
9216 main matmul instructions account for ~2.86 ms of the total.
(fp8 DoubleRow was benchmarked at ~0.7x the per-instruction cost with 2x
contraction, but plain-fp8 quantization fails the 2e-2 gate (1.8e-2
emulated) and hi/lo-split fp8 is time-neutral at the per-instruction
floor, so bf16 is the optimum here.)

Strategy (restructured):
  - Data-parallel: batch B=8192 split across 8 NeuronCores (Bc=1024 each).
  - Feature-major activations on device: every tensor is [H, Bc] so all
    projections are PE matmuls with no on-device transposes.
  - Host-side weight fusion removes chained projections:
        q2  = (Wiq Wq) x_i                            (WQ2)
        dk2 = (Wik Wk[s0]) x_s0 - (Wik Wk[s1]) x_s1   (KA, KB; bik cancels)
        va0 = (Wiv Wv[s0]) x_s0, vb1 = (Wiv Wv[s1]) x_s1
        y   = sum_i (Wout_i Wo_i) ctx_i               (WoC)
    27 HxH matmuls/core -> 18.  All matmuls bf16 (same PE rate as fp32r,
    half the DMA + SBUF), accumulated in fp32 PSUM.
  - Softmax over V-1=2 key views collapses to a sigmoid:
        a0 = sigmoid((q2 . dk2)/sqrt(HD)) per head (head == 128-row tile)
        ctx = vb1 + a0*(va0 - vb1)
  - Everything SBUF-resident: x (3 views, bf16) and ctx tiles stay on chip;
    only a0 (tiny) round-trips DRAM for the partition-broadcast, and the
    final y accumulates into DRAM xacc.
"""

import math

import numpy as np

V = 3
B = 8192
H = 2048
NH = 16
HD = H // NH
EPS = 1e-5
N_CORES = 8
BC = B // N_CORES          # 1024 batch columns per core
NT = H // 128              # 16 h-tiles (== NH heads, HD == 128)
HALF = 512                 # matmul moving free dim
SCALE = 1.0 / math.sqrt(HD)

# others[i] = sources of keys/values for query view i
S0 = [1, 0, 0]
S1 = [2, 2, 1]

# bias-pack rows
BQ, BK, BV0, BV1, BOUT, GAM, BET = 0, 3, 6, 9, 12, 13, 14
NB = 15

_CACHE = {}


def _build_program():
    import concourse.bass as bass
    import concourse.bacc as bacc
    import concourse.tile as tile
    import concourse.mybir as mybir

    f32 = mybir.dt.float32
    f32r = mybir.dt.float32r
    bf16 = mybir.dt.bfloat16
    AF = mybir.ActivationFunctionType
    ALU = mybir.AluOpType

    nc = bacc.Bacc("TRN2", target_bir_lowering=False, debug=False,
                   num_devices=N_CORES)

    # ---- External I/O ----
    xT = nc.dram_tensor("xT", [V, H, BC], bf16, kind="ExternalInput").ap()
    wq2 = nc.dram_tensor("wq2", [V, H, H], bf16, kind="ExternalInput").ap()
    wka = nc.dram_tensor("wka", [V, H, H], bf16, kind="ExternalInput").ap()
    wkb = nc.dram_tensor("wkb", [V, H, H], bf16, kind="ExternalInput").ap()
    wva = nc.dram_tensor("wva", [V, H, H], bf16, kind="ExternalInput").ap()
    wvb = nc.dram_tensor("wvb", [V, H, H], bf16, kind="ExternalInput").ap()
    woc = nc.dram_tensor("woc", [V, H, H], bf16, kind="ExternalInput").ap()
    bpk = nc.dram_tensor("bpk", [NB, 128, NT], f32, kind="ExternalInput").ap()
    onesd = nc.dram_tensor("onesd", [128, 1], f32r, kind="ExternalInput").ap()
    out = nc.dram_tensor("out", [H, BC], f32, kind="ExternalOutput").ap()

    # ---- DRAM scratch ----
    xacc = nc.dram_tensor("xacc", [H, BC], f32r).ap()
    a0_d = nc.dram_tensor("a0_d", [V, NT, BC], bf16).ap()
    ab_d = nc.dram_tensor("ab_d", [2, BC], f32).ap()

    with tile.TileContext(nc) as tc:
        ctxs = []

        def pool(name, bufs, space=None):
            kw = dict(name=name, bufs=bufs)
            if space:
                kw["space"] = space
            p = tc.tile_pool(**kw)
            ctxs.append(p)
            return p.__enter__()

        xin = pool("xin", 1)       # 48 tags x 2KB  (96KB/p)
        ctxp = pool("ctxp", 1)     # 16 tags x 2KB  (32KB/p)
        wp = pool("wp", 1)         # 48 tags x 512B (24KB/p)
        scp = pool("scp", 1)       # q2/dk/pr tags  (12KB/p)
        bcp = pool("bcp", 2)       # bc tag x2      (4KB/p)
        evp = pool("evp", 2)       # ev tag x2      (4KB/p)
        a0p = pool("a0p", 2)       # a0/msq tag x2  (4KB/p)
        lnpa = pool("lnpa", 1)     # ln/nf         (8KB/p)
        lnpb = pool("lnpb", 1)     # sq            (2KB/p)
        lns = pool("lns", 1)       # A/B/mu/m2     (16KB/p)
        cst = pool("cst", 1)       # constants     (~2KB/p)
        psp = pool("psp", 1, space="PSUM")

        # constants
        bias_sb = cst.tile([128, NB, NT], f32)
        nc.sync.dma_start(bias_sb[:], bpk.rearrange("s p f -> p s f"))
        ones_bf = cst.tile([128, 1], bf16)
        nc.vector.memset(ones_bf[:], 1.0)
        ones_r = cst.tile([128, 1], f32r)
        nc.sync.dma_start(ones_r[:], onesd)
        eps_t = cst.tile([1, 1], f32)
        nc.vector.memset(eps_t[:], EPS)

        # resident x tiles: 3 views x 16 h-tiles, bf16
        xs = []
        for v in range(V):
            ts = []
            for t in range(NT):
                tl = xin.tile([128, BC], bf16, tag=f"x{v}_{t}",
                              name=f"x{v}_{t}")
                nc.sync.dma_start(tl[:], xT[v][t * 128:(t + 1) * 128, :])
                ts.append(tl)
            xs.append(ts)

        def ldw(w3, i, gg, tagpfx):
            """Load the 16 h-tiles of weight columns [gg*256,(gg+1)*256)."""
            ws = []
            for ht in range(NT):
                w = wp.tile([128, 256], bf16, tag=f"{tagpfx}{ht}",
                            name=f"{tagpfx}{ht}")
                nc.sync.dma_start(
                    w[:], w3[i][ht * 128:(ht + 1) * 128,
                                gg * 256:(gg + 1) * 256])
                ws.append(w)
            return ws

        for i in range(V):
            s0, s1 = S0[i], S1[i]

            # ===== SCORE phase: q2, dk2, a0 =====
            for gg in range(8):
                wq_t = ldw(wq2, i, gg, "wq")
                wa_t = ldw(wka, i, gg, "wa")
                wb_t = ldw(wkb, i, gg, "wb")
                for gi in range(2):
                    gt = gg * 2 + gi
                    q2t = scp.tile([128, BC], bf16, tag=f"q2_{gi}",
                                   name=f"q2_{gt}")
                    dkt = scp.tile([128, BC], bf16, tag=f"dk_{gi}",
                                   name=f"dk_{gt}")
                    for bh in range(2):
                        sl = slice(bh * HALF, (bh + 1) * HALF)
                        pq = psp.tile([128, HALF], f32, tag=f"p{gi}",
                                      name="pq")
                        for ht in range(NT):
                            nc.tensor.matmul(
                                pq[:], wq_t[ht][:, gi * 128:(gi + 1) * 128],
                                xs[i][ht][:, sl],
                                start=(ht == 0), stop=(ht == NT - 1))
                        pk = psp.tile([128, HALF], f32, tag=f"p{2 + gi}",
                                      name="pk")
                        for ht in range(NT):
                            nc.tensor.matmul(
                                pk[:], wa_t[ht][:, gi * 128:(gi + 1) * 128],
                                xs[s0][ht][:, sl],
                                start=(ht == 0), stop=False)
                        for ht in range(NT):
                            nc.tensor.matmul(
                                pk[:], wb_t[ht][:, gi * 128:(gi + 1) * 128],
                                xs[s1][ht][:, sl],
                                start=False, stop=(ht == NT - 1))
                        nc.scalar.activation(
                            q2t[:, sl], pq[:], AF.Identity,
                            bias=bias_sb[:, BQ + i, gt:gt + 1])
                        nc.scalar.activation(
                            dkt[:, sl], pk[:], AF.Identity,
                            bias=bias_sb[:, BK + i, gt:gt + 1])
                    pr = scp.tile([128, BC], bf16, tag=f"pr{gi}",
                                  name=f"pr{gt}")
                    nc.vector.tensor_tensor(pr[:, 0:HALF], q2t[:, 0:HALF],
                                            dkt[:, 0:HALF], ALU.mult)
                    nc.gpsimd.tensor_tensor(pr[:, HALF:BC], q2t[:, HALF:BC],
                                            dkt[:, HALF:BC], ALU.mult)
                    for bh in range(2):
                        sl = slice(bh * HALF, (bh + 1) * HALF)
                        cs = psp.tile([128, HALF], f32,
                                      tag=f"p{4 + (2 * gi + bh) % 4}",
                                      name="cs")
                        nc.tensor.matmul(cs[0:1, :], ones_bf[:], pr[:, sl],
                                         start=True, stop=True)
                        a0t = a0p.tile([1, HALF], bf16, tag="a0", name="a0t")
                        nc.scalar.activation(a0t[:], cs[0:1, :], AF.Sigmoid,
                                             scale=SCALE)
                        nc.sync.dma_start(a0_d[i, gt:gt + 1, sl], a0t[:])

            # ===== VALUE phase: va0, vb1, ctx =====
            ctx_t = [None] * NT
            for gg in range(8):
                wa_t = ldw(wva, i, gg, "wa")
                wb_t = ldw(wvb, i, gg, "wb")
                for gi in range(2):
                    gt = gg * 2 + gi
                    vat = scp.tile([128, BC], bf16, tag=f"q2_{gi}",
                                   name=f"va_{gt}")
                    vbt = scp.tile([128, BC], bf16, tag=f"dk_{gi}",
                                   name=f"vb_{gt}")
                    for bh in range(2):
                        sl = slice(bh * HALF, (bh + 1) * HALF)
                        pva = psp.tile([128, HALF], f32, tag=f"p{gi}",
                                       name="pva")
                        for ht in range(NT):
                            nc.tensor.matmul(
                                pva[:], wa_t[ht][:, gi * 128:(gi + 1) * 128],
                                xs[s0][ht][:, sl],
                                start=(ht == 0), stop=(ht == NT - 1))
                        pvb = psp.tile([128, HALF], f32, tag=f"p{2 + gi}",
                                       name="pvb")
                        for ht in range(NT):
                            nc.tensor.matmul(
                                pvb[:], wb_t[ht][:, gi * 128:(gi + 1) * 128],
                                xs[s1][ht][:, sl],
                                start=(ht == 0), stop=(ht == NT - 1))
                        nc.scalar.activation(
                            vat[:, sl], pva[:], AF.Identity,
                            bias=bias_sb[:, BV0 + i, gt:gt + 1])
                        nc.scalar.activation(
                            vbt[:, sl], pvb[:], AF.Identity,
                            bias=bias_sb[:, BV1 + i, gt:gt + 1])
                    # ctx = vb1 + a0*(va0 - vb1); a0 broadcast over partitions
                    bct = bcp.tile([128, BC], bf16, tag="bc", name="bct")
                    src = a0_d[i, gt]
                    a0b = bass.AP(tensor=src.tensor, offset=src.offset,
                                  ap=[[0, 128], [1, BC]])
                    nc.sync.dma_start(bct[:], a0b)
                    dv = scp.tile([128, BC], bf16, tag=f"pr{gi}",
                                  name=f"dv{gt}")
                    ct = ctxp.tile([128, BC], bf16, tag=f"c{gt}",
                                   name=f"ctx{gt}")
                    h0 = slice(0, HALF)
                    h1 = slice(HALF, BC)
                    nc.vector.tensor_tensor(dv[:, h0], vat[:, h0], vbt[:, h0],
                                            ALU.subtract)
                    nc.gpsimd.tensor_tensor(dv[:, h1], vat[:, h1], vbt[:, h1],
                                            ALU.subtract)
                    nc.vector.tensor_tensor(dv[:, h0], dv[:, h0], bct[:, h0],
                                            ALU.mult)
                    nc.gpsimd.tensor_tensor(dv[:, h1], dv[:, h1], bct[:, h1],
                                            ALU.mult)
                    nc.vector.tensor_tensor(ct[:, h0], dv[:, h0], vbt[:, h0],
                                            ALU.add)
                    nc.gpsimd.tensor_tensor(ct[:, h1], dv[:, h1], vbt[:, h1],
                                            ALU.add)
                    ctx_t[gt] = ct

            # ===== OUTPUT phase: y += WoC ctx =====
            for gg in range(8):
                wo_t = ldw(woc, i, gg, "wq")
                for gi in range(2):
                    gt = gg * 2 + gi
                    for bh in range(2):
                        sl = slice(bh * HALF, (bh + 1) * HALF)
                        py = psp.tile([128, HALF], f32,
                                      tag=f"p{(2 * gi + bh) % 4}", name="py")
                        for ht in range(NT):
                            nc.tensor.matmul(
                                py[:], wo_t[ht][:, gi * 128:(gi + 1) * 128],
                                ctx_t[ht][:, sl],
                                start=(ht == 0), stop=(ht == NT - 1))
                        et = evp.tile([128, HALF], f32r, tag="ev", name="ev")
                        # all xacc DMA stays on the gpsimd queue: per-queue
                        # FIFO order guarantees init -> accum -> LN read
                        if i == 0:
                            nc.scalar.activation(
                                et[:], py[:], AF.Identity,
                                bias=bias_sb[:, BOUT, gt:gt + 1])
                            nc.gpsimd.dma_start(
                                xacc[gt * 128:(gt + 1) * 128, sl], et[:])
                        else:
                            nc.scalar.activation(et[:], py[:], AF.Copy)
                            nc.gpsimd.dma_start(
                                xacc[gt * 128:(gt + 1) * 128, sl], et[:],
                                accum_op=ALU.add)

        # ===== LN phase: x = x0 + xacc; out = LN(x)*gamma+beta =====
        # pass 1: stats (colsum of x and x^2 over feature dim via PE)
        sx = [psp.tile([128, HALF], f32, tag="p4", name="sx0"),
              psp.tile([128, HALF], f32, tag="p5", name="sx1")]
        sxx = [psp.tile([128, HALF], f32, tag="p6", name="sxx0"),
               psp.tile([128, HALF], f32, tag="p7", name="sxx1")]

        def make_x(t):
            xat = lnpa.tile([128, BC], f32r, tag="ln", name=f"xa{t}")
            nc.gpsimd.dma_start(xat[:], xacc[t * 128:(t + 1) * 128, :])
            nf = lnpa.tile([128, BC], f32r, tag="nf", name=f"nf{t}")
            nc.scalar.activation(nf[:], xs[0][t][:], AF.Copy)
            eng = nc.vector if t % 2 == 0 else nc.gpsimd
            eng.tensor_tensor(nf[:], nf[:], xat[:], ALU.add)
            return nf, eng

        for t in range(NT):
            nf, eng = make_x(t)
            sq = lnpb.tile([128, BC], bf16, tag="sq", name=f"sq{t}")
            eng.tensor_tensor(sq[:], nf[:], nf[:], ALU.mult)
            for hf in range(2):
                sl = slice(hf * HALF, (hf + 1) * HALF)
                nc.tensor.matmul(sx[hf][0:1, :], ones_r[:], nf[:, sl],
                                 start=(t == 0), stop=(t == NT - 1))
                nc.tensor.matmul(sxx[hf][0:1, :], ones_bf[:], sq[:, sl],
                                 start=(t == 0), stop=(t == NT - 1))
        mu = lns.tile([1, BC], f32, tag="mu", name="mu")
        m2 = lns.tile([1, BC], f32, tag="m2", name="m2")
        for hf in range(2):
            sl = slice(hf * HALF, (hf + 1) * HALF)
            nc.scalar.activation(mu[:, sl], sx[hf][0:1, :], AF.Copy,
                                 scale=1.0 / H)
            nc.scalar.activation(m2[:, sl], sxx[hf][0:1, :], AF.Copy,
                                 scale=1.0 / H)
            msq = a0p.tile([1, HALF], f32, tag="a0", name="msq")
            nc.vector.tensor_tensor(msq[:], mu[:, sl], mu[:, sl], ALU.mult)
            nc.vector.tensor_tensor(m2[:, sl], m2[:, sl], msq[:],
                                    ALU.subtract)
        nc.scalar.activation(m2[:], m2[:], AF.Sqrt, bias=eps_t[:])
        nc.vector.reciprocal(m2[:], m2[:])          # rstd
        nc.vector.tensor_tensor(mu[:], mu[:], m2[:], ALU.mult)
        nc.scalar.activation(mu[:], mu[:], AF.Copy, scale=-1.0)  # -mu*rstd
        nc.sync.dma_start(ab_d[0:1, :], m2[:])
        nc.sync.dma_start(ab_d[1:2, :], mu[:])
        A_sb = lns.tile([128, BC], f32, tag="A", name="Asb")
        B_sb = lns.tile([128, BC], f32, tag="B", name="Bsb")
        for r, dst in ((0, A_sb), (1, B_sb)):
            src = ab_d[r]
            bb = bass.AP(tensor=src.tensor, offset=src.offset,
                         ap=[[0, 128], [1, BC]])
            nc.sync.dma_start(dst[:], bb)
        # pass 2: normalize (recompute x = x0 + xacc per tile)
        for t in range(NT):
            nf, eng = make_x(t)
            nff = nf[:].bitcast(f32)
            eng.tensor_tensor(nff, nff, A_sb[:], ALU.mult)
            eng.tensor_tensor(nff, nff, B_sb[:], ALU.add)
            eng.tensor_scalar(
                out=nff, in0=nff,
                scalar1=bias_sb[:, GAM, t:t + 1],
                scalar2=bias_sb[:, BET, t:t + 1],
                op0=ALU.mult, op1=ALU.add)
            nc.sync.dma_start(out[t * 128:(t + 1) * 128, :], nff)

        for p in reversed(ctxs):
            p.__exit__(None, None, None)

    nc.compile()
    return nc


def _prep_host(inputs):
    """Fuse weight pairs (fp32) and pack to bf16 lhsT layout."""
    import ml_dtypes
    bfdt = ml_dtypes.bfloat16

    def f32a(x):
        return np.asarray(x, np.float32)

    views = f32a(inputs["views"])
    Wq, Wk, Wv = f32a(inputs["Wq"]), f32a(inputs["Wk"]), f32a(inputs["Wv"])
    Wiq, Wik, Wiv = f32a(inputs["Wiq"]), f32a(inputs["Wik"]), f32a(inputs["Wiv"])
    Wo, Wout = f32a(inputs["Wo"]), f32a(inputs["Wout"])
    bq, bk, bv = f32a(inputs["bq"]), f32a(inputs["bk"]), f32a(inputs["bv"])
    biq, bik, biv = f32a(inputs["biq"]), f32a(inputs["bik"]), f32a(inputs["biv"])
    bo, bout = f32a(inputs["bo"]), f32a(inputs["bout"])
    gamma, beta = f32a(inputs["gamma"]), f32a(inputs["beta"])

    def lhsT_stack(mats):
        """[V,H,H] bf16 array of W.T per view (lhsT layout [h,g])."""
        a = np.empty((V, H, H), bfdt)
        for i in range(V):
            a[i] = np.ascontiguousarray(mats[i].T).astype(bfdt)
        return a

    wq2 = lhsT_stack([Wiq[i] @ Wq[i] for i in range(V)])
    wka = lhsT_stack([Wik[i] @ Wk[S0[i]] for i in range(V)])
    wkb = lhsT_stack([-(Wik[i] @ Wk[S1[i]]) for i in range(V)])
    wva = lhsT_stack([Wiv[i] @ Wv[S0[i]] for i in range(V)])
    wvb = lhsT_stack([Wiv[i] @ Wv[S1[i]] for i in range(V)])
    Wout_i = [Wout[:, i * H:(i + 1) * H] for i in range(V)]
    woc = lhsT_stack([Wout_i[i] @ Wo[i] for i in range(V)])

    def bcol(vec):
        return np.asarray(vec, np.float32).reshape(NT, 128).T

    bp = np.zeros((NB, 128, NT), np.float32)
    btout = np.asarray(bout, np.float32).copy()
    for i in range(V):
        bp[BQ + i] = bcol(Wiq[i] @ bq[i] + biq[i])
        bp[BK + i] = bcol(Wik[i] @ (bk[S0[i]] - bk[S1[i]]))
        bp[BV0 + i] = bcol(Wiv[i] @ bv[S0[i]] + biv[i])
        bp[BV1 + i] = bcol(Wiv[i] @ bv[S1[i]] + biv[i])
        btout += Wout_i[i] @ bo[i]
    bp[BOUT] = bcol(btout)
    bp[GAM] = bcol(gamma)
    bp[BET] = bcol(beta)

    w = {"wq2": wq2, "wka": wka, "wkb": wkb, "wva": wva, "wvb": wvb,
         "woc": woc, "bpk": bp, "onesd": np.ones((128, 1), np.float32)}

    xts = []
    for c in range(N_CORES):
        sl = views[:, c * BC:(c + 1) * BC, :]
        xts.append(np.ascontiguousarray(sl.transpose(0, 2, 1)).astype(bfdt))
    return w, xts


def kernel(**inputs):
    from concourse.bass_utils import run_bass_kernel_spmd

    trace = bool(_CACHE.get("trace", False))
    if "nc" not in _CACHE:
        _CACHE["nc"] = _build_program()
    nc = _CACHE["nc"]

    w, xts = _prep_host(inputs)
    in_maps = []
    for c in range(N_CORES):
        m = dict(w)
        m["xT"] = xts[c]
        in_maps.append(m)

    res = run_bass_kernel_spmd(nc, in_maps, core_ids=list(range(N_CORES)),
                               trace=trace)
    _CACHE["last_result"] = res

    outp = np.empty((B, H), np.float32)
    for c in range(N_CORES):
        outp[c * BC:(c + 1) * BC, :] = res.results[c]["out"].T
    return outp


# revision 13
# speedup vs baseline: 1.0324x; 1.0324x over previous
"""Trainium2 Bass kernel for nn_CrossAttentionFusion (V=3, B=8192, H=2048, NH=16).

Measured: 2.967 ms HW exec (vs 4.779 ms fp32r baseline), rel err 3.9e-3.
PE-bound at the bf16 1-col/cycle matmul rate with ~96% occupancy; the
9216 main matmul instructions account for ~2.86 ms of the total.
(fp8 DoubleRow measured 262 ns/instr with 2x contraction = 2.37x bf16
work-rate, but plain-fp8 fails the 2e-2 gate (1.8e-2 emulated), the
half-splits fail (2.0-2.1e-2), and the numerically-safe hi/lo full
split needs 24 instr/tile vs bf16's 16 and measured 3.59 ms on HW --
so 18 fused bf16 matmuls is the optimum on this error budget.
All xacc DRAM traffic rides one DMA queue (gpsimd): per-queue FIFO
gives init -> accumulate -> LN-read ordering; cross-queue writes to
the same DRAM scratch raced nondeterministically.)

Strategy (restructured):
  - Data-parallel: batch B=8192 split across 8 NeuronCores (Bc=1024 each).
  - Feature-major activations on device: every tensor is [H, Bc] so all
    projections are PE matmuls with no on-device transposes.
  - Host-side weight fusion removes chained projections:
        q2  = (Wiq Wq) x_i                            (WQ2)
        dk2 = (Wik Wk[s0]) x_s0 - (Wik Wk[s1]) x_s1   (KA, KB; bik cancels)
        va0 = (Wiv Wv[s0]) x_s0, vb1 = (Wiv Wv[s1]) x_s1
        y   = sum_i (Wout_i Wo_i) ctx_i               (WoC)
    27 HxH matmuls/core -> 18.  All matmuls bf16 (same PE rate as fp32r,
    half the DMA + SBUF), accumulated in fp32 PSUM.
  - Softmax over V-1=2 key views collapses to a sigmoid:
        a0 = sigmoid((q2 . dk2)/sqrt(HD)) per head (head == 128-row tile)
        ctx = vb1 + a0*(va0 - vb1)
  - Everything SBUF-resident: x (3 views, bf16) and ctx tiles stay on chip;
    only a0 (tiny) round-trips DRAM for the partition-broadcast, and the
    final y accumulates into DRAM xacc.
"""

import math

import numpy as np

V = 3
B = 8192
H = 2048
NH = 16
HD = H // NH
EPS = 1e-5
N_CORES = 8
BC = B // N_CORES          # 1024 batch columns per core
NT = H // 128              # 16 h-tiles (== NH heads, HD == 128)
HALF = 512                 # matmul moving free dim
SCALE = 1.0 / math.sqrt(HD)

# others[i] = sources of keys/values for query view i
S0 = [1, 0, 0]
S1 = [2, 2, 1]

# bias-pack rows
BQ, BK, BV0, BV1, BOUT, GAM, BET = 0, 3, 6, 9, 12, 13, 14
NB = 15

_CACHE = {}


def _build_program():
    import concourse.bass as bass
    import concourse.bacc as bacc
    import concourse.tile as tile
    import concourse.mybir as mybir

    f32 = mybir.dt.float32
    f32r = mybir.dt.float32r
    bf16 = mybir.dt.bfloat16
    AF = mybir.ActivationFunctionType
    ALU = mybir.AluOpType

    nc = bacc.Bacc("TRN2", target_bir_lowering=False, debug=False,
                   num_devices=N_CORES)

    # ---- External I/O ----
    xT = nc.dram_tensor("xT", [V, H, BC], bf16, kind="ExternalInput").ap()
    wq2 = nc.dram_tensor("wq2", [V, H, H], bf16, kind="ExternalInput").ap()
    wka = nc.dram_tensor("wka", [V, H, H], bf16, kind="ExternalInput").ap()
    wkb = nc.dram_tensor("wkb", [V, H, H], bf16, kind="ExternalInput").ap()
    wva = nc.dram_tensor("wva", [V, H, H], bf16, kind="ExternalInput").ap()
    wvb = nc.dram_tensor("wvb", [V, H, H], bf16, kind="ExternalInput").ap()
    woc = nc.dram_tensor("woc", [V, H, H], bf16, kind="ExternalInput").ap()
    bpk = nc.dram_tensor("bpk", [NB, 128, NT], f32, kind="ExternalInput").ap()
    onesd = nc.dram_tensor("onesd", [128, 1], f32r, kind="ExternalInput").ap()
    out = nc.dram_tensor("out", [H, BC], f32, kind="ExternalOutput").ap()

    # ---- DRAM scratch ----
    xacc = nc.dram_tensor("xacc", [H, BC], f32r).ap()
    a0_d = nc.dram_tensor("a0_d", [V, NT, BC], bf16).ap()
    ab_d = nc.dram_tensor("ab_d", [2, BC], f32).ap()

    with tile.TileContext(nc) as tc:
        ctxs = []

        def pool(name, bufs, space=None):
            kw = dict(name=name, bufs=bufs)
            if space:
                kw["space"] = space
            p = tc.tile_pool(**kw)
            ctxs.append(p)
            return p.__enter__()

        xin = pool("xin", 1)       # 48 tags x 2KB  (96KB/p)
        ctxp = pool("ctxp", 1)     # 16 tags x 2KB  (32KB/p)
        wp = pool("wp", 1)         # 48 tags x 512B (24KB/p)
        scp = pool("scp", 1)       # q2/dk/pr tags  (12KB/p)
        bcp = pool("bcp", 2)       # bc tag x2      (4KB/p)
        evp = pool("evp", 2)       # ev tag x2      (4KB/p)
        a0p = pool("a0p", 2)       # a0/msq tag x2  (4KB/p)
        lnpa = pool("lnpa", 1)     # ln/nf         (8KB/p)
        lnpb = pool("lnpb", 1)     # sq            (2KB/p)
        lns = pool("lns", 1)       # A/B/mu/m2     (16KB/p)
        cst = pool("cst", 1)       # constants     (~2KB/p)
        psp = pool("psp", 1, space="PSUM")

        # constants
        bias_sb = cst.tile([128, NB, NT], f32)
        nc.sync.dma_start(bias_sb[:], bpk.rearrange("s p f -> p s f"))
        ones_bf = cst.tile([128, 1], bf16)
        nc.vector.memset(ones_bf[:], 1.0)
        ones_r = cst.tile([128, 1], f32r)
        nc.sync.dma_start(ones_r[:], onesd)
        eps_t = cst.tile([1, 1], f32)
        nc.vector.memset(eps_t[:], EPS)

        # resident x tiles: 3 views x 16 h-tiles, bf16
        xs = []
        for v in range(V):
            ts = []
            for t in range(NT):
                tl = xin.tile([128, BC], bf16, tag=f"x{v}_{t}",
                              name=f"x{v}_{t}")
                nc.sync.dma_start(tl[:], xT[v][t * 128:(t + 1) * 128, :])
                ts.append(tl)
            xs.append(ts)

        def ldw(w3, i, gg, tagpfx):
            """Load the 16 h-tiles of weight columns [gg*256,(gg+1)*256)."""
            ws = []
            for ht in range(NT):
                w = wp.tile([128, 256], bf16, tag=f"{tagpfx}{ht}",
                            name=f"{tagpfx}{ht}")
                nc.sync.dma_start(
                    w[:], w3[i][ht * 128:(ht + 1) * 128,
                                gg * 256:(gg + 1) * 256])
                ws.append(w)
            return ws

        for i in range(V):
            s0, s1 = S0[i], S1[i]

            # ===== SCORE phase: q2, dk2, a0 =====
            for gg in range(8):
                wq_t = ldw(wq2, i, gg, "wq")
                wa_t = ldw(wka, i, gg, "wa")
                wb_t = ldw(wkb, i, gg, "wb")
                for gi in range(2):
                    gt = gg * 2 + gi
                    q2t = scp.tile([128, BC], bf16, tag=f"q2_{gi}",
                                   name=f"q2_{gt}")
                    dkt = scp.tile([128, BC], bf16, tag=f"dk_{gi}",
                                   name=f"dk_{gt}")
                    for bh in range(2):
                        sl = slice(bh * HALF, (bh + 1) * HALF)
                        pq = psp.tile([128, HALF], f32, tag=f"p{gi}",
                                      name="pq")
                        for ht in range(NT):
                            nc.tensor.matmul(
                                pq[:], wq_t[ht][:, gi * 128:(gi + 1) * 128],
                                xs[i][ht][:, sl],
                                start=(ht == 0), stop=(ht == NT - 1))
                        pk = psp.tile([128, HALF], f32, tag=f"p{2 + gi}",
                                      name="pk")
                        for ht in range(NT):
                            nc.tensor.matmul(
                                pk[:], wa_t[ht][:, gi * 128:(gi + 1) * 128],
                                xs[s0][ht][:, sl],
                                start=(ht == 0), stop=False)
                        for ht in range(NT):
                            nc.tensor.matmul(
                                pk[:], wb_t[ht][:, gi * 128:(gi + 1) * 128],
                                xs[s1][ht][:, sl],
                                start=False, stop=(ht == NT - 1))
                        nc.scalar.activation(
                            q2t[:, sl], pq[:], AF.Identity,
                            bias=bias_sb[:, BQ + i, gt:gt + 1])
                        nc.scalar.activation(
                            dkt[:, sl], pk[:], AF.Identity,
                            bias=bias_sb[:, BK + i, gt:gt + 1])
                    pr = scp.tile([128, BC], bf16, tag=f"pr{gi}",
                                  name=f"pr{gt}")
                    nc.vector.tensor_tensor(pr[:, 0:HALF], q2t[:, 0:HALF],
                                            dkt[:, 0:HALF], ALU.mult)
                    nc.gpsimd.tensor_tensor(pr[:, HALF:BC], q2t[:, HALF:BC],
                                            dkt[:, HALF:BC], ALU.mult)
                    for bh in range(2):
                        sl = slice(bh * HALF, (bh + 1) * HALF)
                        cs = psp.tile([128, HALF], f32,
                                      tag=f"p{4 + (2 * gi + bh) % 4}",
                                      name="cs")
                        nc.tensor.matmul(cs[0:1, :], ones_bf[:], pr[:, sl],
                                         start=True, stop=True)
                        a0t = a0p.tile([1, HALF], bf16, tag="a0", name="a0t")
                        nc.scalar.activation(a0t[:], cs[0:1, :], AF.Sigmoid,
                                             scale=SCALE)
                        nc.sync.dma_start(a0_d[i, gt:gt + 1, sl], a0t[:])

            # ===== VALUE phase: va0, vb1, ctx =====
            ctx_t = [None] * NT
            for gg in range(8):
                wa_t = ldw(wva, i, gg, "wa")
                wb_t = ldw(wvb, i, gg, "wb")
                for gi in range(2):
                    gt = gg * 2 + gi
                    vat = scp.tile([128, BC], bf16, tag=f"q2_{gi}",
                                   name=f"va_{gt}")
                    vbt = scp.tile([128, BC], bf16, tag=f"dk_{gi}",
                                   name=f"vb_{gt}")
                    for bh in range(2):
                        sl = slice(bh * HALF, (bh + 1) * HALF)
                        pva = psp.tile([128, HALF], f32, tag=f"p{gi}",
                                       name="pva")
                        for ht in range(NT):
                            nc.tensor.matmul(
                                pva[:], wa_t[ht][:, gi * 128:(gi + 1) * 128],
                                xs[s0][ht][:, sl],
                                start=(ht == 0), stop=(ht == NT - 1))
                        pvb = psp.tile([128, HALF], f32, tag=f"p{2 + gi}",
                                       name="pvb")
                        for ht in range(NT):
                            nc.tensor.matmul(
                                pvb[:], wb_t[ht][:, gi * 128:(gi + 1) * 128],
                                xs[s1][ht][:, sl],
                                start=(ht == 0), stop=(ht == NT - 1))
                        nc.scalar.activation(
                            vat[:, sl], pva[:], AF.Identity,
                            bias=bias_sb[:, BV0 + i, gt:gt + 1])
                        nc.scalar.activation(
                            vbt[:, sl], pvb[:], AF.Identity,
                            bias=bias_sb[:, BV1 + i, gt:gt + 1])
                    # ctx = vb1 + a0*(va0 - vb1); a0 broadcast over partitions
                    bct = bcp.tile([128, BC], bf16, tag="bc", name="bct")
                    src = a0_d[i, gt]
                    a0b = bass.AP(tensor=src.tensor, offset=src.offset,
                                  ap=[[0, 128], [1, BC]])
                    nc.sync.dma_start(bct[:], a0b)
                    dv = scp.tile([128, BC], bf16, tag=f"pr{gi}",
                                  name=f"dv{gt}")
                    ct = ctxp.tile([128, BC], bf16, tag=f"c{gt}",
                                   name=f"ctx{gt}")
                    h0 = slice(0, HALF)
                    h1 = slice(HALF, BC)
                    nc.vector.tensor_tensor(dv[:, h0], vat[:, h0], vbt[:, h0],
                                            ALU.subtract)
                    nc.gpsimd.tensor_tensor(dv[:, h1], vat[:, h1], vbt[:, h1],
                                            ALU.subtract)
                    nc.vector.tensor_tensor(dv[:, h0], dv[:, h0], bct[:, h0],
                                            ALU.mult)
                    nc.gpsimd.tensor_tensor(dv[:, h1], dv[:, h1], bct[:, h1],
                                            ALU.mult)
                    nc.vector.tensor_tensor(ct[:, h0], dv[:, h0], vbt[:, h0],
                                            ALU.add)
                    nc.gpsimd.tensor_tensor(ct[:, h1], dv[:, h1], vbt[:, h1],
                                            ALU.add)
                    ctx_t[gt] = ct

            # ===== OUTPUT phase: y += WoC ctx =====
            for gg in range(8):
                wo_t = ldw(woc, i, gg, "wq")
                for gi in range(2):
                    gt = gg * 2 + gi
                    for bh in range(2):
                        sl = slice(bh * HALF, (bh + 1) * HALF)
                        py = psp.tile([128, HALF], f32,
                                      tag=f"p{(2 * gi + bh) % 4}", name="py")
                        for ht in range(NT):
                            nc.tensor.matmul(
                                py[:], wo_t[ht][:, gi * 128:(gi + 1) * 128],
                                ctx_t[ht][:, sl],
                                start=(ht == 0), stop=(ht == NT - 1))
                        et = evp.tile([128, HALF], f32r, tag="ev", name="ev")
                        # all xacc DMA stays on the gpsimd queue: per-queue
                        # FIFO order guarantees init -> accum -> LN read
                        if i == 0:
                            nc.scalar.activation(
                                et[:], py[:], AF.Identity,
                                bias=bias_sb[:, BOUT, gt:gt + 1])
                            nc.gpsimd.dma_start(
                                xacc[gt * 128:(gt + 1) * 128, sl], et[:])
                        else:
                            nc.scalar.activation(et[:], py[:], AF.Copy)
                            nc.gpsimd.dma_start(
                                xacc[gt * 128:(gt + 1) * 128, sl], et[:],
                                accum_op=ALU.add)

        # ===== LN phase: x = x0 + xacc; out = LN(x)*gamma+beta =====
        # pass 1: stats (colsum of x and x^2 over feature dim via PE)
        sx = [psp.tile([128, HALF], f32, tag="p4", name="sx0"),
              psp.tile([128, HALF], f32, tag="p5", name="sx1")]
        sxx = [psp.tile([128, HALF], f32, tag="p6", name="sxx0"),
               psp.tile([128, HALF], f32, tag="p7", name="sxx1")]

        def make_x(t):
            xat = lnpa.tile([128, BC], f32r, tag="ln", name=f"xa{t}")
            nc.gpsimd.dma_start(xat[:], xacc[t * 128:(t + 1) * 128, :])
            nf = lnpa.tile([128, BC], f32r, tag="nf", name=f"nf{t}")
            nc.scalar.activation(nf[:], xs[0][t][:], AF.Copy)
            eng = nc.vector if t % 2 == 0 else nc.gpsimd
            eng.tensor_tensor(nf[:], nf[:], xat[:], ALU.add)
            return nf, eng

        for t in range(NT):
            nf, eng = make_x(t)
            sq = lnpb.tile([128, BC], bf16, tag="sq", name=f"sq{t}")
            eng.tensor_tensor(sq[:], nf[:], nf[:], ALU.mult)
            for hf in range(2):
                sl = slice(hf * HALF, (hf + 1) * HALF)
                nc.tensor.matmul(sx[hf][0:1, :], ones_r[:], nf[:, sl],
                                 start=(t == 0), stop=(t == NT - 1))
                nc.tensor.matmul(sxx[hf][0:1, :], ones_bf[:], sq[:, sl],
                                 start=(t == 0), stop=(t == NT - 1))
        mu = lns.tile([1, BC], f32, tag="mu", name="mu")
        m2 = lns.tile([1, BC], f32, tag="m2", name="m2")
        for hf in range(2):
            sl = slice(hf * HALF, (hf + 1) * HALF)
            nc.scalar.activation(mu[:, sl], sx[hf][0:1, :], AF.Copy,
                                 scale=1.0 / H)
            nc.scalar.activation(m2[:, sl], sxx[hf][0:1, :], AF.Copy,
                                 scale=1.0 / H)
            msq = a0p.tile([1, HALF], f32, tag="a0", name="msq")
            nc.vector.tensor_tensor(msq[:], mu[:, sl], mu[:, sl], ALU.mult)
            nc.vector.tensor_tensor(m2[:, sl], m2[:, sl], msq[:],
                                    ALU.subtract)
        nc.scalar.activation(m2[:], m2[:], AF.Sqrt, bias=eps_t[:])
        nc.vector.reciprocal(m2[:], m2[:])          # rstd
        nc.vector.tensor_tensor(mu[:], mu[:], m2[:], ALU.mult)
        nc.scalar.activation(mu[:], mu[:], AF.Copy, scale=-1.0)  # -mu*rstd
        nc.sync.dma_start(ab_d[0:1, :], m2[:])
        nc.sync.dma_start(ab_d[1:2, :], mu[:])
        A_sb = lns.tile([128, BC], f32, tag="A", name="Asb")
        B_sb = lns.tile([128, BC], f32, tag="B", name="Bsb")
        for r, dst in ((0, A_sb), (1, B_sb)):
            src = ab_d[r]
            bb = bass.AP(tensor=src.tensor, offset=src.offset,
                         ap=[[0, 128], [1, BC]])
            nc.sync.dma_start(dst[:], bb)
        # pass 2: normalize (recompute x = x0 + xacc per tile)
        for t in range(NT):
            nf, eng = make_x(t)
            nff = nf[:].bitcast(f32)
            eng.tensor_tensor(nff, nff, A_sb[:], ALU.mult)
            eng.tensor_tensor(nff, nff, B_sb[:], ALU.add)
            eng.tensor_scalar(
                out=nff, in0=nff,
                scalar1=bias_sb[:, GAM, t:t + 1],
                scalar2=bias_sb[:, BET, t:t + 1],
                op0=ALU.mult, op1=ALU.add)
            nc.sync.dma_start(out[t * 128:(t + 1) * 128, :], nff)

        for p in reversed(ctxs):
            p.__exit__(None, None, None)

    nc.compile()
    return nc


def _prep_host(inputs):
    """Fuse weight pairs (fp32) and pack to bf16 lhsT layout."""
    import ml_dtypes
    bfdt = ml_dtypes.bfloat16

    def f32a(x):
        return np.asarray(x, np.float32)

    views = f32a(inputs["views"])
    Wq, Wk, Wv = f32a(inputs["Wq"]), f32a(inputs["Wk"]), f32a(inputs["Wv"])
    Wiq, Wik, Wiv = f32a(inputs["Wiq"]), f32a(inputs["Wik"]), f32a(inputs["Wiv"])
    Wo, Wout = f32a(inputs["Wo"]), f32a(inputs["Wout"])
    bq, bk, bv = f32a(inputs["bq"]), f32a(inputs["bk"]), f32a(inputs["bv"])
    biq, bik, biv = f32a(inputs["biq"]), f32a(inputs["bik"]), f32a(inputs["biv"])
    bo, bout = f32a(inputs["bo"]), f32a(inputs["bout"])
    gamma, beta = f32a(inputs["gamma"]), f32a(inputs["beta"])

    def lhsT_stack(mats):
        """[V,H,H] bf16 array of W.T per view (lhsT layout [h,g])."""
        a = np.empty((V, H, H), bfdt)
        for i in range(V):
            a[i] = np.ascontiguousarray(mats[i].T).astype(bfdt)
        return a

    wq2 = lhsT_stack([Wiq[i] @ Wq[i] for i in range(V)])
    wka = lhsT_stack([Wik[i] @ Wk[S0[i]] for i in range(V)])
    wkb = lhsT_stack([-(Wik[i] @ Wk[S1[i]]) for i in range(V)])
    wva = lhsT_stack([Wiv[i] @ Wv[S0[i]] for i in range(V)])
    wvb = lhsT_stack([Wiv[i] @ Wv[S1[i]] for i in range(V)])
    Wout_i = [Wout[:, i * H:(i + 1) * H] for i in range(V)]
    woc = lhsT_stack([Wout_i[i] @ Wo[i] for i in range(V)])

    def bcol(vec):
        return np.asarray(vec, np.float32).reshape(NT, 128).T

    bp = np.zeros((NB, 128, NT), np.float32)
    btout = np.asarray(bout, np.float32).copy()
    for i in range(V):
        bp[BQ + i] = bcol(Wiq[i] @ bq[i] + biq[i])
        bp[BK + i] = bcol(Wik[i] @ (bk[S0[i]] - bk[S1[i]]))
        bp[BV0 + i] = bcol(Wiv[i] @ bv[S0[i]] + biv[i])
        bp[BV1 + i] = bcol(Wiv[i] @ bv[S1[i]] + biv[i])
        btout += Wout_i[i] @ bo[i]
    bp[BOUT] = bcol(btout)
    bp[GAM] = bcol(gamma)
    bp[BET] = bcol(beta)

    w = {"wq2": wq2, "wka": wka, "wkb": wkb, "wva": wva, "wvb": wvb,
         "woc": woc, "bpk": bp, "onesd": np.ones((128, 1), np.float32)}

    xts = []
    for c in range(N_CORES):
        sl = views[:, c * BC:(c + 1) * BC, :]
        xts.append(np.ascontiguousarray(sl.transpose(0, 2, 1)).astype(bfdt))
    return w, xts


def kernel(**inputs):
    from concourse.bass_utils import run_bass_kernel_spmd

    trace = bool(_CACHE.get("trace", False))
    if "nc" not in _CACHE:
        _CACHE["nc"] = _build_program()
    nc = _CACHE["nc"]

    w, xts = _prep_host(inputs)
    in_maps = []
    for c in range(N_CORES):
        m = dict(w)
        m["xT"] = xts[c]
        in_maps.append(m)

    res = run_bass_kernel_spmd(nc, in_maps, core_ids=list(range(N_CORES)),
                               trace=trace)
    _CACHE["last_result"] = res

    outp = np.empty((B, H), np.float32)
    for c in range(N_CORES):
        outp[c * BC:(c + 1) * BC, :] = res.results[c]["out"].T
    return outp
